# revision 9
# baseline (speedup 1.0000x reference)
"""Trainium2 Bass kernel for AttnProcessor self-attention (B=2,S=2048,C=1024,H=16).

Sharding: 8 cores, core c owns heads (2c, 2c+1) for both batches (tensor
parallel on the head dim for QKV). Projections run in fp8e4 DoubleRow
(hs and x32-scaled weights; the ones-column carries 32.0 so softmax
normalization absorbs the scale); QK runs bf16; PV runs fp8 DoubleRow
(probs e5m2, v e4m3); output projection bf16.

Token->core output mapping is interleaved so each 512-token q-slice (qs)
contains one 64-token block for every destination core: core c owns tokens
512*qs + 64*c .. +64 of every (b, qs). Attention output is NORMALIZED
SENDER-SIDE (reciprocal of the softmax sums is broadcast across partitions
via a tiny K=1 matmul, then fused into the psum->bf16 drain cast), so each
(b, qs) ships a [8,128,64] bf16 AllToAll right after its PV completes and
the receive side is a single DMA straight into the outproj input tile.
Output projection runs per qs-pair as PE fill work inside later attention
slices / the tail.

b0's hs arrives in 512-token column slices so the projection pipeline
starts after ~1/4 of the load. ScalarE runs ONLY exps; DMAs are issued
from sync/vector/gpsimd rings; DVE does casts/recips/biases/normalization.
"""
import numpy as np

import concourse.bacc as bacc
import concourse.bass as bass
import concourse.tile as tile
import concourse.tile_rust as tile_rust
from concourse import mybir
from concourse.bass_utils import run_bass_kernel_spmd

F32 = mybir.dt.float32
BF16 = mybir.dt.bfloat16
FP8E4 = mybir.dt.float8e4
FP8E5 = mybir.dt.float8e5

B, S, C, H, D = 2, 2048, 1024, 16, 64
N_CORES = 8
BS = B * S  # 4096
SCALE = 1.0 / np.sqrt(D)
WS = 32.0  # fp8 projection weight scale (weights are sigma=1/32)

# a2a chunk geometry (per destination core): 128 rows x 64 tokens bf16
CH_BF = 128 * 64          # bf16 elems per dest chunk (8192)

_CACHE = {}
DR = mybir.MatmulPerfMode.DoubleRow


def _build():
    nc = bacc.Bacc(num_devices=N_CORES)
    hsT = nc.declare_dram_parameter("hsT", [C, BS], FP8E4, isOutput=False)
    wq = nc.declare_dram_parameter("wq", [128, 1024], FP8E4, isOutput=False)
    wk = nc.declare_dram_parameter("wk", [128, 1024], FP8E4, isOutput=False)
    wv = nc.declare_dram_parameter("wv", [128, 1152], FP8E4, isOutput=False)
    wo = nc.declare_dram_parameter("wo", [C, C], BF16, isOutput=False)
    bqk = nc.declare_dram_parameter("bqk", [128, 2], F32, isOutput=False)
    bvb = nc.declare_dram_parameter("bvb", [1, 130], F32, isOutput=False)
    onesb = nc.declare_dram_parameter("onesb", [1, 64], BF16, isOutput=False)
    res = nc.declare_dram_parameter("res", [512, C], BF16, isOutput=False)
    out1 = nc.declare_dram_parameter("out1", [256, C], F32, isOutput=True)
    out2 = nc.declare_dram_parameter("out2", [256, C], F32, isOutput=True)

    with tile.TileContext(nc) as tc:
        with (
            tc.tile_pool(name="wpool", bufs=1) as wpool,
            tc.tile_pool(name="hpool", bufs=1) as hpool,
            tc.tile_pool(name="qkpool", bufs=2) as qkpool,
            tc.tile_pool(name="ppool", bufs=4) as ppool,
            tc.tile_pool(name="spool", bufs=4) as spool,
            tc.tile_pool(name="opool", bufs=2) as opool,
            tc.tile_pool(name="psum", bufs=1, space="PSUM") as psum,
            tc.tile_pool(name="dram", bufs=1, space="DRAM") as dram,
        ):
            # ---- weight / constant loads ----
            wq_sb = wpool.tile([128, 1024], FP8E4, tag="wq")
            nc.sync.dma_start(out=wq_sb[:], in_=wq[:])
            wk_sb = wpool.tile([128, 1024], FP8E4, tag="wk")
            nc.scalar.dma_start(out=wk_sb[:], in_=wk[:])
            wv_sb = wpool.tile([128, 1152], FP8E4, tag="wv")
            nc.gpsimd.dma_start(out=wv_sb[:], in_=wv[:])
            bqk_sb = wpool.tile([128, 2], F32, tag="bqk")
            nc.gpsimd.dma_start(out=bqk_sb[:], in_=bqk[:])
            ones_sb = wpool.tile([1, 64], BF16, tag="ones")
            nc.sync.dma_start(out=ones_sb[:], in_=onesb[:])
            bvb_sb = wpool.tile([128, 130], F32, tag="bvb")
            bvb_ap = bvb[:]
            nc.gpsimd.dma_start(
                out=bvb_sb[:],
                in_=bass.AP(tensor=bvb_ap.tensor, offset=bvb_ap.offset,
                            ap=[[0, 128], [1, 130]]))

            # b0 hs: chunk-pair tiles [128, 2x512] per (mp, j4), one DMA
            # each, streamed token-slice-major across 3 rings
            rings = [nc.sync, nc.scalar, nc.gpsimd]
            hsT_ap = hsT[:]
            hs0 = [[None] * 4 for _ in range(4)]
            hs0_dmas = []
            n = 0
            for j4 in range(4):
                for mp in range(4):
                    t = hpool.tile([128, 1024], FP8E4, tag=f"hs0_{mp}_{j4}",
                                   name=f"hs0_{mp}_{j4}")
                    tp = t[:]
                    d = rings[n % 3].dma_start(
                        out=bass.AP(tensor=tp.tensor, offset=tp.offset,
                                    ap=[list(tp.ap[0]), [512, 2], [1, 512]]),
                        in_=bass.AP(tensor=hsT_ap.tensor,
                                    offset=hsT_ap.offset
                                    + 128 * 2 * mp * BS + 512 * j4,
                                    ap=[[BS, 128], [128 * BS, 2], [1, 512]]))
                    hs0_dmas.append(d)
                    n += 1
                    hs0[mp][j4] = t
            hs1 = []
            for mp in range(4):
                t = hpool.tile([128, 4096], FP8E4, tag=f"hs1_{mp}",
                               name=f"hs1_{mp}")
                tp = t[:]
                d = [nc.sync, nc.scalar][mp % 2].dma_start(
                    out=bass.AP(tensor=tp.tensor, offset=tp.offset,
                                ap=[list(tp.ap[0]), [2048, 2], [1, 2048]]),
                    in_=bass.AP(tensor=hsT_ap.tensor,
                                offset=hsT_ap.offset
                                + 128 * 2 * mp * BS + 2048,
                                ap=[[BS, 128], [128 * BS, 2], [1, 2048]]))
                for a in hs0_dmas[-3:]:
                    tile_rust.add_dep_helper(
                        d.ins, a.ins, True, "hs1 after hs0 (bandwidth)")
                hs1.append(t)

            a2a_in = [[dram.tile([8, 128, 64], BF16, name=f"a2ain{b}_{qs}")
                       for qs in range(4)] for b in range(2)]
            a2a_out = [[dram.tile([8, 128, 64], BF16, name=f"a2aout{b}_{qs}")
                        for qs in range(4)] for b in range(2)]

            qT, kT, vS = {}, {}, {}

            def hs_rhs(b, mp, col, width):
                """fp8 DR AP for hs chunk-pair mp, token cols [col,col+w)."""
                if b == 0:
                    t = hs0[mp][col // 512][:]
                    off = col % 512
                    return bass.AP(tensor=t.tensor, offset=t.offset + off,
                                   ap=[list(t.ap[0]), [512, 2], [1, width]])
                t = hs1[mp][:]
                return bass.AP(tensor=t.tensor, offset=t.offset + col,
                               ap=[list(t.ap[0]), [2048, 2], [1, width]])

            def emit_proj_qk(b, t_idx, j2):
                """One unit: tensor t_idx (0=q,1=k), one 256-wide s-slice.
                fp8 DoubleRow over chunk pairs."""
                if t_idx == 0:
                    if b not in qT:
                        qT[b] = qkpool.tile([128, 2048], BF16, tag="qT",
                                            name=f"qT{b}")
                    dst, w_sb = qT[b], wq_sb
                else:
                    if b not in kT:
                        kT[b] = qkpool.tile([128, 2048], BF16, tag="kT",
                                            name=f"kT{b}")
                    dst, w_sb = kT[b], wk_sb
                ps = psum.tile([128, 512], F32, tag="aux", bufs=2,
                               name=f"pqk{b}_{t_idx}_{j2}")
                sl = ps[:, 0:256]
                wap = w_sb[:]
                for mp in range(4):
                    nc.tensor.matmul(
                        sl,
                        bass.AP(tensor=wap.tensor,
                                offset=wap.offset + 256 * mp,
                                ap=[list(wap.ap[0]), [128, 2], [1, 128]]),
                        hs_rhs(b, mp, 256 * j2, 256),
                        start=(mp == 0), stop=(mp == 3), perf_mode=DR)
                nc.vector.tensor_scalar_add(
                    out=dst[:, 256 * j2:256 * (j2 + 1)], in0=sl,
                    scalar1=bqk_sb[:, t_idx:t_idx + 1])

            def emit_proj_v(b, i):
                """One unit: one 128-row v' s-tile i -> fp8e4 vS (x32).
                vS layout (PV DoubleRow pairs): pair kc'=i//2 block at
                320*kc', head h at +160*h, parity i%2 at +80."""
                if b not in vS:
                    vS[b] = qkpool.tile([128, 2560], FP8E4, tag="vS",
                                        name=f"vS{b}")
                ps = psum.tile([128, 512], F32, tag="aux", bufs=2,
                               name=f"pv{b}_{i}")
                sl = ps[:, 0:144]
                wap = wv_sb[:]
                for mp in range(4):
                    nc.tensor.matmul(
                        sl,
                        hs_rhs(b, mp, 128 * i, 128),
                        bass.AP(tensor=wap.tensor,
                                offset=wap.offset + 288 * mp,
                                ap=[list(wap.ap[0]), [144, 2], [1, 144]]),
                        start=(mp == 0), stop=(mp == 3), perf_mode=DR)
                vt = vS[b][:]
                bvt = bvb_sb[:]
                slb = ps[:, 0:130]
                nc.vector.tensor_tensor(
                    out=bass.AP(tensor=vt.tensor, offset=vt.offset
                                + 320 * (i // 2) + 80 * (i % 2),
                                ap=[list(vt.ap[0]), [160, 2], [1, 65]]),
                    in0=bass.AP(tensor=slb.tensor, offset=slb.offset,
                                ap=[list(slb.ap[0]), [65, 2], [1, 65]]),
                    in1=bass.AP(tensor=bvt.tensor, offset=bvt.offset,
                                ap=[list(bvt.ap[0]), [65, 2], [1, 65]]),
                    op=mybir.AluOpType.add)

            def emit_attention_qs(b, qs, fill_work):
                """One q-slice (512 q) for both heads; 16 kc steps.
                Per step: 2 fills, exp(kc), QK(kc+1); PV (DoubleRow,
                paired kc blocks) after odd kc."""
                accA = psum.tile([65, 512], F32, tag="accA", bufs=1,
                                 name=f"accA_{b}_{qs}")
                accB = psum.tile([65, 512], F32, tag="accB", bufs=1,
                                 name=f"accB_{b}_{qs}")
                sc_t = {}

                def emit_qk(kc):
                    sc = psum.tile([128, 1024], F32, tag="sc", bufs=2,
                                   name=f"sc_{b}_{qs}_{kc}")
                    sc_t[kc] = sc
                    nc.tensor.matmul(
                        sc[:, 0:512],
                        kT[b][0:64, 128 * kc:128 * (kc + 1)],
                        qT[b][0:64, 512 * qs:512 * (qs + 1)],
                        start=True, stop=True)
                    nc.tensor.matmul(
                        sc[:, 512:1024],
                        kT[b][64:128, 128 * kc:128 * (kc + 1)],
                        qT[b][64:128, 512 * qs:512 * (qs + 1)],
                        start=True, stop=True)

                emit_qk(0)
                pr2 = None
                for kc in range(16):
                    for _ in range(2):
                        if fill_work:
                            fill_work.pop(0)()
                    if kc % 2 == 0:
                        pr2 = ppool.tile([128, 2048], FP8E5, tag="pr",
                                         bufs=2, name=f"pr_{b}_{qs}_{kc}")
                    nc.scalar.activation(
                        pr2[:, 1024 * (kc % 2):1024 * (kc % 2 + 1)],
                        sc_t.pop(kc)[:],
                        mybir.ActivationFunctionType.Exp,
                        scale=float(SCALE / (WS * WS)))
                    if kc < 15:
                        emit_qk(kc + 1)
                    if kc % 2 == 1:
                        kp = kc // 2
                        vt = vS[b][:]
                        prt = pr2[:]
                        for h, acc in ((0, accA), (1, accB)):
                            nc.tensor.matmul(
                                acc[:],
                                bass.AP(tensor=vt.tensor, offset=vt.offset
                                        + 320 * kp + 160 * h,
                                        ap=[list(vt.ap[0]), [80, 2],
                                            [1, 65]]),
                                bass.AP(tensor=prt.tensor,
                                        offset=prt.offset + 512 * h,
                                        ap=[list(prt.ap[0]), [1024, 2],
                                            [1, 512]]),
                                start=(kp == 0), stop=(kp == 7),
                                perf_mode=DR)

                # drain: reciprocal of sums (bf16), broadcast across 64
                # partitions via K=1 matmul, normalize+cast in one DVE op
                # per head, ship per-dest chunks right away.
                sm2 = spool.tile([1, 1024], F32, tag="sm2",
                                 name=f"sm2_{b}_{qs}")
                nc.vector.tensor_copy(sm2[:, 0:512], accA[64:65, :])
                nc.vector.tensor_copy(sm2[:, 512:1024], accB[64:65, :])
                rb2f = spool.tile([1, 1024], F32, tag="rbf",
                                  name=f"rbf_{b}_{qs}")
                nc.vector.reciprocal_approx_fast(rb2f[:], sm2[:])
                rb2 = spool.tile([1, 1024], BF16, tag="rb",
                                 name=f"rb_{b}_{qs}")
                nc.vector.tensor_copy(rb2[:], rb2f[:])
                bc = psum.tile([128, 1024], F32, tag="sc", bufs=2,
                               name=f"bc_{b}_{qs}")
                nc.tensor.matmul(bc[0:64, 0:512], ones_sb[:],
                                 rb2[:, 0:512], start=True, stop=True)
                nc.tensor.matmul(bc[0:64, 512:1024], ones_sb[:],
                                 rb2[:, 512:1024], start=True, stop=True)
                bcb = spool.tile([64, 1024], BF16, tag="bcb",
                                 name=f"bcb_{b}_{qs}")
                nc.vector.tensor_copy(bcb[:], bc[0:64, :])
                a2a_t = a2a_in[b][qs][:]
                peng = nc.gpsimd if b == 0 else nc.sync
                st = spool.tile([64, 1024], BF16, tag="st",
                                name=f"st_{b}_{qs}")
                stp = st[:]
                for h, acc in ((0, accA), (1, accB)):
                    nc.vector.tensor_tensor(
                        out=st[:, 512 * h:512 * (h + 1)],
                        in0=acc[0:64, :],
                        in1=bcb[:, 512 * h:512 * (h + 1)],
                        op=mybir.AluOpType.mult)
                    # payload: st[0:64, 512h + 64*d+t] -> a2a[d, 64h+r, t]
                    peng.dma_start(
                        out=bass.AP(tensor=a2a_t.tensor, offset=a2a_t.offset
                                    + 64 * h * 64,
                                    ap=[[64, 64], [CH_BF, 8], [1, 64]]),
                        in_=bass.AP(tensor=stp.tensor,
                                    offset=stp.offset + 512 * h,
                                    ap=[list(stp.ap[0]), [64, 8], [1, 64]]))
                nc.gpsimd.collective_compute(
                    "AllToAll", mybir.AluOpType.bypass,
                    replica_groups=[list(range(8))],
                    ins=[a2a_in[b][qs][:]], outs=[a2a_out[b][qs][:]])

            # ---- output side ----
            an_all = {}

            def emit_recv(b, p, half):
                """After A2A (b, qs=2p+half): one DMA into the an tile."""
                qs = 2 * p + half
                a2a_t = a2a_out[b][qs][:]
                if (b, p) not in an_all:
                    an_all[(b, p)] = opool.tile([128, 1024], BF16, tag="an",
                                                name=f"an{b}_{p}")
                anap = an_all[(b, p)][:]
                raw_d = nc.sync.dma_start(
                    out=bass.AP(tensor=anap.tensor,
                                offset=anap.offset + 64 * half,
                                ap=[list(anap.ap[0]), [128, 8], [1, 64]]),
                    in_=bass.AP(tensor=a2a_t.tensor, offset=a2a_t.offset,
                                ap=[[64, 128], [CH_BF, 8], [1, 64]]))
                return raw_d

            wo_sb = []
            res_sb = []

            out_ps = {}

            def emit_out_mm(b, p, co):
                """Outproj half matmuls: 512 cols for 128 tokens of pair."""
                an = an_all[(b, p)]
                ps = psum.tile([128, 512], F32, tag="aux", bufs=2,
                               name=f"op{b}_{p}_{co}")
                for j in range(8):
                    nc.tensor.matmul(
                        ps[:], an[:, 128 * j:128 * (j + 1)],
                        wo_sb[j][:, 512 * co:512 * (co + 1)],
                        start=(j == 0), stop=(j == 7))
                out_ps[(b, p, co)] = ps

            def emit_out_fin(b, p, co):
                """Residual add + store (DVE+sync). Emitted late so the
                psum read never sits ahead of drain casts in the DVE FIFO."""
                ps = out_ps.pop((b, p, co))
                ob = opool.tile([128, 512], F32, tag="ob",
                                name=f"ob{b}_{p}_{co}")
                nc.vector.tensor_tensor(
                    out=ob[:], in0=ps[:],
                    in1=res_sb[2 * b + p][:, 512 * co:512 * (co + 1)],
                    op=mybir.AluOpType.add)
                out_t = out1 if b == 0 else out2
                nc.sync.dma_start(
                    out=out_t[128 * p:128 * (p + 1),
                              512 * co:512 * (co + 1)],
                    in_=ob[:])

            # ---------------- emission ----------------
            emit_proj_qk(0, 0, 0)
            emit_proj_qk(0, 0, 1)
            emit_proj_qk(0, 1, 0)
            emit_proj_v(0, 0)
            emit_proj_v(0, 1)

            def qk_u(b, t, j2):
                return lambda: emit_proj_qk(b, t, j2)

            def v_u(b, i):
                return lambda: emit_proj_v(b, i)

            def nop():
                pass

            fill = [qk_u(0, 1, 1), v_u(0, 2),
                    qk_u(0, 1, 2), v_u(0, 3),
                    qk_u(0, 1, 3), v_u(0, 4),
                    qk_u(0, 1, 4), v_u(0, 5),
                    qk_u(0, 1, 5), v_u(0, 6),
                    qk_u(0, 1, 6), v_u(0, 7),
                    qk_u(0, 1, 7), v_u(0, 8),
                    v_u(0, 9), v_u(0, 10),
                    v_u(0, 11), v_u(0, 12),
                    v_u(0, 13), v_u(0, 14),
                    v_u(0, 15), qk_u(0, 0, 2),
                    qk_u(0, 0, 3)]
            emit_attention_qs(0, 0, fill)
            assert not fill

            # wo / res load (sync queue)
            for cc in range(8):
                t = hpool.tile([128, 1024], BF16, tag=f"wo{cc}",
                               name=f"wo{cc}")
                nc.sync.dma_start(out=t[:],
                                  in_=wo[128 * cc:128 * (cc + 1), :])
                wo_sb.append(t)
            for st_i in range(4):
                t = wpool.tile([128, 1024], BF16, tag=f"res{st_i}",
                               name=f"res{st_i}")
                nc.sync.dma_start(out=t[:],
                                  in_=res[128 * st_i:128 * (st_i + 1), :])
                res_sb.append(t)

            # b1 projections fill b0 qs1-qs3 (hs1 lands ~35us in)
            fill = [qk_u(0, 0, 4), qk_u(0, 0, 5)]
            for j2 in range(8):
                fill.append(qk_u(1, 1, j2))
            fill += [v_u(1, 0), v_u(1, 1)]
            emit_attention_qs(0, 1, fill)
            fill = [qk_u(0, 0, 6), qk_u(0, 0, 7)]
            for i in range(2, 8):
                fill.append(v_u(1, i))
            fill += [qk_u(1, 0, 0), qk_u(1, 0, 1), qk_u(1, 0, 2),
                     qk_u(1, 0, 3)]
            emit_attention_qs(0, 2, fill)
            fill = [qk_u(1, 0, 4), qk_u(1, 0, 5), qk_u(1, 0, 6),
                    qk_u(1, 0, 7)]
            for i in range(8, 16):
                fill.append(v_u(1, i))
            emit_attention_qs(0, 3, fill)

            emit_attention_qs(1, 0, [])
            emit_attention_qs(1, 1, [])
            emit_recv(0, 0, 0)
            emit_recv(0, 0, 1)
            fill = [nop] * 10 + [lambda: emit_out_mm(0, 0, 0),
                                 lambda: emit_out_mm(0, 0, 1)]
            emit_attention_qs(1, 2, fill)
            emit_out_fin(0, 0, 0)
            emit_out_fin(0, 0, 1)
            emit_recv(0, 1, 0)
            emit_recv(0, 1, 1)
            fill = [nop] * 10 + [lambda: emit_out_mm(0, 1, 0),
                                 lambda: emit_out_mm(0, 1, 1)]
            emit_attention_qs(1, 3, fill)
            emit_out_fin(0, 1, 0)
            emit_out_fin(0, 1, 1)
            # tail: out(1,0) doubles as PE warm-keeper during last A2A
            emit_recv(1, 0, 0)
            emit_recv(1, 0, 1)
            emit_out_mm(1, 0, 0)
            emit_out_fin(1, 0, 0)
            emit_out_mm(1, 0, 1)
            emit_out_fin(1, 0, 1)
            raw_d = emit_recv(1, 1, 0)
            warm = psum.tile([128, 512], F32, tag="aux", bufs=2,
                             name="warm")
            for wi in range(6):
                w = nc.tensor.matmul(warm[:], wo_sb[0][:, 0:128],
                                     wo_sb[1][:, 0:512],
                                     start=True, stop=True,
                                     skip_group_check=True)
                if wi == 0:
                    tile_rust.add_dep_helper(
                        w.ins, raw_d.ins, True, "warm PE near last recv")
            emit_recv(1, 1, 1)
            emit_out_mm(1, 1, 0)
            emit_out_fin(1, 1, 0)
            emit_out_mm(1, 1, 1)
            emit_out_fin(1, 1, 1)
    nc.finalize()
    return nc


def _prep_inputs(hidden_states, Wq, bq, Wk, bk, Wv, bv, Wo, bo):
    import ml_dtypes
    bf16 = ml_dtypes.bfloat16
    fp8 = ml_dtypes.float8_e4m3fn
    hs = np.asarray(hidden_states, np.float32)
    hsT = np.clip(np.ascontiguousarray(
        hs.transpose(2, 0, 1).reshape(C, BS)), -240, 240).astype(fp8)
    Wo_h = np.ascontiguousarray(np.asarray(Wo, np.float32)).astype(bf16)
    bo_f = np.asarray(bo, np.float32)
    ones64 = np.ones((1, 64), np.float32).astype(bf16)

    def pack_pairs(w, ncols, stride):
        """[C, ncols] -> [128, 8*stride]: col 2*stride*mp + stride*i + m
        = WS * w[128*(2*mp+i) + p, m], fp8."""
        out = np.zeros((128, 8 * stride), np.float32)
        for mp in range(4):
            for i in range(2):
                blk = w[128 * (2 * mp + i):128 * (2 * mp + i + 1), :]
                out[:, 2 * stride * mp + stride * i:
                    2 * stride * mp + stride * i + ncols] = WS * blk
        return np.clip(out, -240, 240).astype(fp8)

    in_maps = []
    for c in range(N_CORES):
        h0 = 2 * c
        cols = slice(64 * h0, 64 * h0 + 128)
        wv_c = np.zeros((C, 130), np.float32)
        bvb_c = np.zeros((1, 130), np.float32)
        for a in range(2):
            hd = slice(64 * (h0 + a), 64 * (h0 + a + 1))
            wv_c[:, 65 * a:65 * a + 64] = np.asarray(Wv, np.float32)[:, hd]
            bvb_c[0, 65 * a:65 * a + 64] = WS * np.asarray(
                bv, np.float32)[hd]
            bvb_c[0, 65 * a + 64] = WS  # ones column x32: sums match v x32
        bqk_c = WS * np.stack([np.asarray(bq, np.float32)[cols],
                               np.asarray(bk, np.float32)[cols]], axis=1)
        res_c = np.empty((512, C), np.float32)
        for b in range(2):
            for qs in range(4):
                rows = slice(64 * (4 * b + qs), 64 * (4 * b + qs) + 64)
                toks = slice(512 * qs + 64 * c, 512 * qs + 64 * c + 64)
                res_c[rows] = hs[b, toks, :] + bo_f
        in_maps.append({
            "hsT": hsT,
            "wq": pack_pairs(np.asarray(Wq, np.float32)[:, cols], 128, 128),
            "wk": pack_pairs(np.asarray(Wk, np.float32)[:, cols], 128, 128),
            "wv": pack_pairs(wv_c, 130, 144),
            "wo": Wo_h,
            "bqk": np.ascontiguousarray(bqk_c),
            "bvb": bvb_c,
            "onesb": ones64,
            "res": np.ascontiguousarray(res_c).astype(bf16),
        })
    return in_maps


def _run(inputs, trace=False, trace_kwargs=None):
    if "nc" not in _CACHE:
        _CACHE["nc"] = _build()
    nc = _CACHE["nc"]
    in_maps = _prep_inputs(**inputs)
    r = run_bass_kernel_spmd(nc, in_maps, core_ids=list(range(N_CORES)),
                             trace=trace, **(trace_kwargs or {}))
    full = np.empty((B, S, C), np.float32)
    for c in range(N_CORES):
        for b in range(2):
            o = r.results[c]["out1" if b == 0 else "out2"]
            for qs in range(4):
                full[b, 512 * qs + 64 * c:512 * qs + 64 * c + 64, :] = \
                    o[64 * qs:64 * qs + 64]
    return full, r


def kernel(**inputs):
    full, _ = _run(inputs, trace=False)
    return full


# revision 14
# speedup vs baseline: 1.0369x; 1.0369x over previous
"""Trainium2 Bass kernel for AttnProcessor self-attention (B=2,S=2048,C=1024,H=16).

Sharding: 8 cores, core c owns heads (2c, 2c+1) for both batches (tensor
parallel on the head dim for QKV). Projections run in fp8e4 DoubleRow
(hs and x32-scaled weights; the ones-column carries 32.0 so softmax
normalization absorbs the scale); QK runs bf16; PV runs fp8 DoubleRow
(probs e5m2, v e4m3); output projection bf16.

Token->core output mapping is interleaved so each 512-token q-slice (qs)
contains one 64-token block for every destination core: core c owns tokens
512*qs + 64*c .. +64 of every (b, qs). Attention output is NORMALIZED
SENDER-SIDE (reciprocal of the softmax sums is broadcast across partitions
via a tiny K=1 matmul, then fused into the psum->bf16 drain cast), so each
(b, qs) ships a [8,128,64] bf16 AllToAll right after its PV completes and
the receive side is a single DMA straight into the outproj input tile.
Output projection runs per qs-pair as PE fill work inside later attention
slices / the tail.

b0's hs arrives in 512-token column slices so the projection pipeline
starts after ~1/4 of the load. ScalarE runs ONLY exps; DMAs are issued
from sync/vector/gpsimd rings; DVE does casts/recips/biases/normalization.
"""
import numpy as np

import concourse.bacc as bacc
import concourse.bass as bass
import concourse.tile as tile
import concourse.tile_rust as tile_rust
from concourse import mybir
from concourse.bass_utils import run_bass_kernel_spmd

F32 = mybir.dt.float32
BF16 = mybir.dt.bfloat16
FP8E4 = mybir.dt.float8e4
FP8E5 = mybir.dt.float8e5

B, S, C, H, D = 2, 2048, 1024, 16, 64
N_CORES = 8
BS = B * S  # 4096
SCALE = 1.0 / np.sqrt(D)
WS = 32.0  # fp8 projection weight scale (weights are sigma=1/32)

# a2a chunk geometry (per destination core): 128 rows x 64 tokens bf16
CH_BF = 128 * 64          # bf16 elems per dest chunk (8192)

_CACHE = {}
DR = mybir.MatmulPerfMode.DoubleRow


def _build():
    nc = bacc.Bacc(num_devices=N_CORES)
    hsT = nc.declare_dram_parameter("hsT", [C, BS], FP8E4, isOutput=False)
    wq = nc.declare_dram_parameter("wq", [128, 1024], FP8E4, isOutput=False)
    wk = nc.declare_dram_parameter("wk", [128, 1024], FP8E4, isOutput=False)
    wv = nc.declare_dram_parameter("wv", [128, 1152], FP8E4, isOutput=False)
    wo = nc.declare_dram_parameter("wo", [C, C], BF16, isOutput=False)
    bqk = nc.declare_dram_parameter("bqk", [128, 2], F32, isOutput=False)
    bvb = nc.declare_dram_parameter("bvb", [1, 130], F32, isOutput=False)
    onesb = nc.declare_dram_parameter("onesb", [1, 64], BF16, isOutput=False)
    res = nc.declare_dram_parameter("res", [512, C], BF16, isOutput=False)
    out1 = nc.declare_dram_parameter("out1", [256, C], F32, isOutput=True)
    out2 = nc.declare_dram_parameter("out2", [256, C], F32, isOutput=True)

    with tile.TileContext(nc) as tc:
        with (
            tc.tile_pool(name="wpool", bufs=1) as wpool,
            tc.tile_pool(name="hpool", bufs=1) as hpool,
            tc.tile_pool(name="qkpool", bufs=2) as qkpool,
            tc.tile_pool(name="ppool", bufs=4) as ppool,
            tc.tile_pool(name="spool", bufs=4) as spool,
            tc.tile_pool(name="opool", bufs=2) as opool,
            tc.tile_pool(name="psum", bufs=1, space="PSUM") as psum,
            tc.tile_pool(name="dram", bufs=1, space="DRAM") as dram,
        ):
            # ---- weight / constant loads ----
            wq_sb = wpool.tile([128, 1024], FP8E4, tag="wq")
            nc.sync.dma_start(out=wq_sb[:], in_=wq[:])
            wk_sb = wpool.tile([128, 1024], FP8E4, tag="wk")
            nc.scalar.dma_start(out=wk_sb[:], in_=wk[:])
            wv_sb = wpool.tile([128, 1152], FP8E4, tag="wv")
            nc.gpsimd.dma_start(out=wv_sb[:], in_=wv[:])
            bqk_sb = wpool.tile([128, 2], F32, tag="bqk")
            nc.gpsimd.dma_start(out=bqk_sb[:], in_=bqk[:])
            ones_sb = wpool.tile([1, 64], BF16, tag="ones")
            nc.sync.dma_start(out=ones_sb[:], in_=onesb[:])
            bvb_sb = wpool.tile([128, 130], F32, tag="bvb")
            bvb_ap = bvb[:]
            nc.gpsimd.dma_start(
                out=bvb_sb[:],
                in_=bass.AP(tensor=bvb_ap.tensor, offset=bvb_ap.offset,
                            ap=[[0, 128], [1, 130]]))

            # b0 hs: chunk-pair tiles [128, 2x512] per (mp, j4), one DMA
            # each, streamed token-slice-major across 3 rings
            rings = [nc.sync, nc.scalar, nc.gpsimd]
            hsT_ap = hsT[:]
            hs0 = [[None] * 4 for _ in range(4)]
            hs0_dmas = []
            n = 0
            for j4 in range(4):
                for mp in range(4):
                    t = hpool.tile([128, 1024], FP8E4, tag=f"hs0_{mp}_{j4}",
                                   name=f"hs0_{mp}_{j4}")
                    for i in range(2):
                        d = rings[n % 3].dma_start(
                            out=t[:, 512 * i:512 * (i + 1)],
                            in_=hsT[128 * (2 * mp + i):
                                    128 * (2 * mp + i + 1),
                                    512 * j4:512 * (j4 + 1)])
                        hs0_dmas.append(d)
                        n += 1
                    hs0[mp][j4] = t
            hs1 = []
            for mp in range(4):
                t = hpool.tile([128, 4096], FP8E4, tag=f"hs1_{mp}",
                               name=f"hs1_{mp}")
                tp = t[:]
                d = [nc.sync, nc.scalar][mp % 2].dma_start(
                    out=bass.AP(tensor=tp.tensor, offset=tp.offset,
                                ap=[list(tp.ap[0]), [2048, 2], [1, 2048]]),
                    in_=bass.AP(tensor=hsT_ap.tensor,
                                offset=hsT_ap.offset
                                + 128 * 2 * mp * BS + 2048,
                                ap=[[BS, 128], [128 * BS, 2], [1, 2048]]))
                for a in hs0_dmas[-3:]:
                    tile_rust.add_dep_helper(
                        d.ins, a.ins, True, "hs1 after hs0 (bandwidth)")
                hs1.append(t)

            a2a_in = [[dram.tile([8, 128, 64], BF16, name=f"a2ain{b}_{qs}")
                       for qs in range(4)] for b in range(2)]
            a2a_out = [[dram.tile([8, 128, 64], BF16, name=f"a2aout{b}_{qs}")
                        for qs in range(4)] for b in range(2)]

            qT, kT, vS = {}, {}, {}
            _dr = {}

            def hs_rhs(b, mp, col, width):
                """fp8 DR AP for hs chunk-pair mp, token cols [col,col+w)."""
                if b == 0:
                    t = hs0[mp][col // 512][:]
                    off = col % 512
                    return bass.AP(tensor=t.tensor, offset=t.offset + off,
                                   ap=[list(t.ap[0]), [512, 2], [1, width]])
                t = hs1[mp][:]
                return bass.AP(tensor=t.tensor, offset=t.offset + col,
                               ap=[list(t.ap[0]), [2048, 2], [1, width]])

            def emit_proj_qk(b, t_idx, j2):
                """One unit: tensor t_idx (0=q,1=k), one 256-wide s-slice.
                fp8 DoubleRow over chunk pairs."""
                if t_idx == 0:
                    if b not in qT:
                        qT[b] = qkpool.tile([128, 2048], BF16, tag="qT",
                                            name=f"qT{b}")
                    dst, w_sb = qT[b], wq_sb
                else:
                    if b not in kT:
                        kT[b] = qkpool.tile([128, 2048], BF16, tag="kT",
                                            name=f"kT{b}")
                    dst, w_sb = kT[b], wk_sb
                ps = psum.tile([128, 512], F32, tag="aux", bufs=2,
                               name=f"pqk{b}_{t_idx}_{j2}")
                sl = ps[:, 0:256]
                wap = w_sb[:]
                for mp in range(4):
                    nc.tensor.matmul(
                        sl,
                        bass.AP(tensor=wap.tensor,
                                offset=wap.offset + 256 * mp,
                                ap=[list(wap.ap[0]), [128, 2], [1, 128]]),
                        hs_rhs(b, mp, 256 * j2, 256),
                        start=(mp == 0), stop=(mp == 3), perf_mode=DR)
                nc.vector.tensor_scalar_add(
                    out=dst[:, 256 * j2:256 * (j2 + 1)], in0=sl,
                    scalar1=bqk_sb[:, t_idx:t_idx + 1])

            def emit_proj_v(b, i):
                """One unit: one 128-row v' s-tile i -> fp8e4 vS (x32).
                vS layout (PV DoubleRow pairs): pair kc'=i//2 block at
                320*kc', head h at +160*h, parity i%2 at +80."""
                if b not in vS:
                    vS[b] = qkpool.tile([128, 2560], FP8E4, tag="vS",
                                        name=f"vS{b}")
                ps = psum.tile([128, 512], F32, tag="aux", bufs=2,
                               name=f"pv{b}_{i}")
                sl = ps[:, 0:144]
                wap = wv_sb[:]
                for mp in range(4):
                    nc.tensor.matmul(
                        sl,
                        hs_rhs(b, mp, 128 * i, 128),
                        bass.AP(tensor=wap.tensor,
                                offset=wap.offset + 288 * mp,
                                ap=[list(wap.ap[0]), [144, 2], [1, 144]]),
                        start=(mp == 0), stop=(mp == 3), perf_mode=DR)
                vt = vS[b][:]
                bvt = bvb_sb[:]
                slb = ps[:, 0:130]
                nc.vector.tensor_tensor(
                    out=bass.AP(tensor=vt.tensor, offset=vt.offset
                                + 320 * (i // 2) + 80 * (i % 2),
                                ap=[list(vt.ap[0]), [160, 2], [1, 65]]),
                    in0=bass.AP(tensor=slb.tensor, offset=slb.offset,
                                ap=[list(slb.ap[0]), [65, 2], [1, 65]]),
                    in1=bass.AP(tensor=bvt.tensor, offset=bvt.offset,
                                ap=[list(bvt.ap[0]), [65, 2], [1, 65]]),
                    op=mybir.AluOpType.add)

            def emit_attention_qs(b, qs, fill_work):
                """One q-slice (512 q) for both heads; 16 kc steps.
                Per step: 2 fills, exp(kc), QK(kc+1); PV (DoubleRow,
                paired kc blocks) after odd kc."""
                accA = psum.tile([65, 512], F32, tag="accA", bufs=1,
                                 name=f"accA_{b}_{qs}")
                accB = psum.tile([65, 512], F32, tag="accB", bufs=1,
                                 name=f"accB_{b}_{qs}")
                sc_t = {}

                def emit_qk(kc):
                    sc = psum.tile([128, 1024], F32, tag="sc", bufs=2,
                                   name=f"sc_{b}_{qs}_{kc}")
                    sc_t[kc] = sc
                    nc.tensor.matmul(
                        sc[:, 0:512],
                        kT[b][0:64, 128 * kc:128 * (kc + 1)],
                        qT[b][0:64, 512 * qs:512 * (qs + 1)],
                        start=True, stop=True)
                    nc.tensor.matmul(
                        sc[:, 512:1024],
                        kT[b][64:128, 128 * kc:128 * (kc + 1)],
                        qT[b][64:128, 512 * qs:512 * (qs + 1)],
                        start=True, stop=True)

                emit_qk(0)
                pr2 = None
                for kc in range(16):
                    for _ in range(2):
                        if fill_work:
                            fill_work.pop(0)()
                    if kc % 2 == 0:
                        pr2 = ppool.tile([128, 2048], FP8E5, tag="pr",
                                         bufs=2, name=f"pr_{b}_{qs}_{kc}")
                    nc.scalar.activation(
                        pr2[:, 1024 * (kc % 2):1024 * (kc % 2 + 1)],
                        sc_t.pop(kc)[:],
                        mybir.ActivationFunctionType.Exp,
                        scale=float(SCALE / (WS * WS)))
                    if kc < 15:
                        emit_qk(kc + 1)
                    if kc % 2 == 1:
                        kp = kc // 2
                        vt = vS[b][:]
                        prt = pr2[:]
                        for h, acc in ((0, accA), (1, accB)):
                            nc.tensor.matmul(
                                acc[:],
                                bass.AP(tensor=vt.tensor, offset=vt.offset
                                        + 320 * kp + 160 * h,
                                        ap=[list(vt.ap[0]), [80, 2],
                                            [1, 65]]),
                                bass.AP(tensor=prt.tensor,
                                        offset=prt.offset + 512 * h,
                                        ap=[list(prt.ap[0]), [1024, 2],
                                            [1, 512]]),
                                start=(kp == 0), stop=(kp == 7),
                                perf_mode=DR)

                # drain (returned as deferred closures so the PE-queue
                # bcast matmul never stalls the next slice's QK stream):
                # reciprocal of sums (bf16), broadcast across 64 partitions
                # via K=1 matmul, normalize+cast in one DVE op per head,
                # ship per-dest chunks.
                def drain0(accA=accA, accB=accB, b=b, qs=qs):
                    sm2 = spool.tile([1, 1024], F32, tag="sm2",
                                     name=f"sm2_{b}_{qs}")
                    nc.vector.tensor_copy(sm2[:, 0:512], accA[64:65, :])
                    nc.vector.tensor_copy(sm2[:, 512:1024], accB[64:65, :])
                    rb2f = spool.tile([1, 1024], F32, tag="rbf",
                                      name=f"rbf_{b}_{qs}")
                    nc.vector.reciprocal_approx_fast(rb2f[:], sm2[:])
                    rb2 = spool.tile([1, 1024], BF16, tag="rb",
                                     name=f"rb_{b}_{qs}")
                    nc.vector.tensor_copy(rb2[:], rb2f[:])
                    bc = psum.tile([128, 1024], F32, tag="sc", bufs=2,
                                   name=f"bc_{b}_{qs}")
                    nc.tensor.matmul(bc[0:64, 0:512], ones_sb[:],
                                     rb2[:, 0:512], start=True, stop=True)
                    nc.tensor.matmul(bc[0:64, 512:1024], ones_sb[:],
                                     rb2[:, 512:1024], start=True, stop=True)
                    bcb = spool.tile([64, 1024], BF16, tag="bcb",
                                     name=f"bcb_{b}_{qs}")
                    nc.vector.tensor_copy(bcb[:], bc[0:64, :])
                    st = spool.tile([64, 1024], BF16, tag="st",
                                    name=f"st_{b}_{qs}")
                    _dr[(b, qs)] = (bcb, st)

                def drain_h(h, acc, b=b, qs=qs):
                    bcb, st = _dr[(b, qs)]
                    a2a_t = a2a_in[b][qs][:]
                    stp = st[:]
                    peng = nc.gpsimd if b == 0 else nc.sync
                    nc.vector.tensor_tensor(
                        out=st[:, 512 * h:512 * (h + 1)],
                        in0=acc[0:64, :],
                        in1=bcb[:, 512 * h:512 * (h + 1)],
                        op=mybir.AluOpType.mult)
                    # payload: st[0:64, 512h + 64*d+t] -> a2a[d, 64h+r, t]
                    peng.dma_start(
                        out=bass.AP(tensor=a2a_t.tensor, offset=a2a_t.offset
                                    + 64 * h * 64,
                                    ap=[[64, 64], [CH_BF, 8], [1, 64]]),
                        in_=bass.AP(tensor=stp.tensor,
                                    offset=stp.offset + 512 * h,
                                    ap=[list(stp.ap[0]), [64, 8], [1, 64]]))
                    if h == 1:
                        nc.gpsimd.collective_compute(
                            "AllToAll", mybir.AluOpType.bypass,
                            replica_groups=[list(range(8))],
                            ins=[a2a_in[b][qs][:]],
                            outs=[a2a_out[b][qs][:]])

                return [drain0,
                        lambda: drain_h(0, accA),
                        lambda: drain_h(1, accB)]

            # ---- output side ----
            an_all = {}

            def emit_recv(b, p, half):
                """After A2A (b, qs=2p+half): one DMA into the an tile."""
                qs = 2 * p + half
                a2a_t = a2a_out[b][qs][:]
                if (b, p) not in an_all:
                    an_all[(b, p)] = opool.tile([128, 1024], BF16, tag="an",
                                                name=f"an{b}_{p}")
                anap = an_all[(b, p)][:]
                raw_d = nc.sync.dma_start(
                    out=bass.AP(tensor=anap.tensor,
                                offset=anap.offset + 64 * half,
                                ap=[list(anap.ap[0]), [128, 8], [1, 64]]),
                    in_=bass.AP(tensor=a2a_t.tensor, offset=a2a_t.offset,
                                ap=[[64, 128], [CH_BF, 8], [1, 64]]))
                return raw_d

            wo_sb = []
            res_sb = []

            out_ps = {}

            def emit_out_mm(b, p, co):
                """Outproj half matmuls: 512 cols for 128 tokens of pair."""
                an = an_all[(b, p)]
                ps = psum.tile([128, 512], F32, tag="aux", bufs=2,
                               name=f"op{b}_{p}_{co}")
                for j in range(8):
                    nc.tensor.matmul(
                        ps[:], an[:, 128 * j:128 * (j + 1)],
                        wo_sb[j][:, 512 * co:512 * (co + 1)],
                        start=(j == 0), stop=(j == 7))
                out_ps[(b, p, co)] = ps

            def emit_out_fin(b, p, co):
                """Residual add + store (DVE+sync). Emitted late so the
                psum read never sits ahead of drain casts in the DVE FIFO."""
                ps = out_ps.pop((b, p, co))
                ob = opool.tile([128, 512], F32, tag="ob",
                                name=f"ob{b}_{p}_{co}")
                nc.vector.tensor_tensor(
                    out=ob[:], in0=ps[:],
                    in1=res_sb[2 * b + p][:, 512 * co:512 * (co + 1)],
                    op=mybir.AluOpType.add)
                out_t = out1 if b == 0 else out2
                nc.sync.dma_start(
                    out=out_t[128 * p:128 * (p + 1),
                              512 * co:512 * (co + 1)],
                    in_=ob[:])

            # ---------------- emission ----------------
            emit_proj_qk(0, 0, 0)
            emit_proj_qk(0, 0, 1)
            emit_proj_qk(0, 1, 0)
            emit_proj_v(0, 0)
            emit_proj_v(0, 1)

            def qk_u(b, t, j2):
                return lambda: emit_proj_qk(b, t, j2)

            def v_u(b, i):
                return lambda: emit_proj_v(b, i)

            def nop():
                pass

            fill = [qk_u(0, 1, 1), v_u(0, 2),
                    qk_u(0, 1, 2), v_u(0, 3),
                    qk_u(0, 1, 3), v_u(0, 4),
                    qk_u(0, 1, 4), v_u(0, 5),
                    qk_u(0, 1, 5), v_u(0, 6),
                    qk_u(0, 1, 6), v_u(0, 7),
                    qk_u(0, 1, 7), v_u(0, 8),
                    v_u(0, 9), v_u(0, 10),
                    v_u(0, 11), v_u(0, 12),
                    v_u(0, 13), v_u(0, 14),
                    v_u(0, 15), qk_u(0, 0, 2),
                    qk_u(0, 0, 3)]
            dr = emit_attention_qs(0, 0, fill)
            assert not fill

            # wo / res load (sync queue)
            for cc in range(8):
                t = hpool.tile([128, 1024], BF16, tag=f"wo{cc}",
                               name=f"wo{cc}")
                nc.sync.dma_start(out=t[:],
                                  in_=wo[128 * cc:128 * (cc + 1), :])
                wo_sb.append(t)
            for st_i in range(4):
                t = wpool.tile([128, 1024], BF16, tag=f"res{st_i}",
                               name=f"res{st_i}")
                nc.sync.dma_start(out=t[:],
                                  in_=res[128 * st_i:128 * (st_i + 1), :])
                res_sb.append(t)

            # b1 projections fill b0 qs1-qs3 (hs1 lands ~35us in)
            fill = dr + [qk_u(0, 0, 4), qk_u(0, 0, 5)]
            for j2 in range(8):
                fill.append(qk_u(1, 1, j2))
            fill += [v_u(1, 0), v_u(1, 1)]
            dr = emit_attention_qs(0, 1, fill)
            fill = dr + [qk_u(0, 0, 6), qk_u(0, 0, 7)]
            for i in range(2, 8):
                fill.append(v_u(1, i))
            fill += [qk_u(1, 0, 0), qk_u(1, 0, 1), qk_u(1, 0, 2),
                     qk_u(1, 0, 3)]
            dr = emit_attention_qs(0, 2, fill)
            fill = dr + [qk_u(1, 0, 4), qk_u(1, 0, 5), qk_u(1, 0, 6),
                         qk_u(1, 0, 7)]
            for i in range(8, 16):
                fill.append(v_u(1, i))
            dr = emit_attention_qs(0, 3, fill)

            dr = emit_attention_qs(1, 0, list(dr))
            dr = emit_attention_qs(1, 1, list(dr))
            emit_recv(0, 0, 0)
            emit_recv(0, 0, 1)
            fill = dr + [nop] * 8 + [lambda: emit_out_mm(0, 0, 0),
                                     lambda: emit_out_mm(0, 0, 1)]
            dr = emit_attention_qs(1, 2, fill)
            emit_out_fin(0, 0, 0)
            emit_out_fin(0, 0, 1)
            emit_recv(0, 1, 0)
            emit_recv(0, 1, 1)
            fill = dr + [nop] * 8 + [lambda: emit_out_mm(0, 1, 0),
                                     lambda: emit_out_mm(0, 1, 1)]
            dr = emit_attention_qs(1, 3, fill)
            # last drain runs immediately: its A2A is the critical path
            for f in dr:
                f()
            emit_out_fin(0, 1, 0)
            emit_out_fin(0, 1, 1)
            # tail: out(1,0) doubles as PE warm-keeper during last A2A
            emit_recv(1, 0, 0)
            emit_recv(1, 0, 1)
            emit_out_mm(1, 0, 0)
            emit_out_fin(1, 0, 0)
            emit_out_mm(1, 0, 1)
            emit_out_fin(1, 0, 1)
            raw_d = emit_recv(1, 1, 0)
            warm = psum.tile([128, 512], F32, tag="aux", bufs=2,
                             name="warm")
            for wi in range(6):
                w = nc.tensor.matmul(warm[:], wo_sb[0][:, 0:128],
                                     wo_sb[1][:, 0:512],
                                     start=True, stop=True,
                                     skip_group_check=True)
                if wi == 0:
                    tile_rust.add_dep_helper(
                        w.ins, raw_d.ins, True, "warm PE near last recv")
            emit_recv(1, 1, 1)
            emit_out_mm(1, 1, 0)
            emit_out_fin(1, 1, 0)
            emit_out_mm(1, 1, 1)
            emit_out_fin(1, 1, 1)
    nc.finalize()
    return nc


def _prep_inputs(hidden_states, Wq, bq, Wk, bk, Wv, bv, Wo, bo):
    import ml_dtypes
    bf16 = ml_dtypes.bfloat16
    fp8 = ml_dtypes.float8_e4m3fn
    hs = np.asarray(hidden_states, np.float32)
    hsT = np.clip(np.ascontiguousarray(
        hs.transpose(2, 0, 1).reshape(C, BS)), -240, 240).astype(fp8)
    Wo_h = np.ascontiguousarray(np.asarray(Wo, np.float32)).astype(bf16)
    bo_f = np.asarray(bo, np.float32)
    ones64 = np.ones((1, 64), np.float32).astype(bf16)

    def pack_pairs(w, ncols, stride):
        """[C, ncols] -> [128, 8*stride]: col 2*stride*mp + stride*i + m
        = WS * w[128*(2*mp+i) + p, m], fp8."""
        out = np.zeros((128, 8 * stride), np.float32)
        for mp in range(4):
            for i in range(2):
                blk = w[128 * (2 * mp + i):128 * (2 * mp + i + 1), :]
                out[:, 2 * stride * mp + stride * i:
                    2 * stride * mp + stride * i + ncols] = WS * blk
        return np.clip(out, -240, 240).astype(fp8)

    in_maps = []
    for c in range(N_CORES):
        h0 = 2 * c
        cols = slice(64 * h0, 64 * h0 + 128)
        wv_c = np.zeros((C, 130), np.float32)
        bvb_c = np.zeros((1, 130), np.float32)
        for a in range(2):
            hd = slice(64 * (h0 + a), 64 * (h0 + a + 1))
            wv_c[:, 65 * a:65 * a + 64] = np.asarray(Wv, np.float32)[:, hd]
            bvb_c[0, 65 * a:65 * a + 64] = WS * np.asarray(
                bv, np.float32)[hd]
            bvb_c[0, 65 * a + 64] = WS  # ones column x32: sums match v x32
        bqk_c = WS * np.stack([np.asarray(bq, np.float32)[cols],
                               np.asarray(bk, np.float32)[cols]], axis=1)
        res_c = np.empty((512, C), np.float32)
        for b in range(2):
            for qs in range(4):
                rows = slice(64 * (4 * b + qs), 64 * (4 * b + qs) + 64)
                toks = slice(512 * qs + 64 * c, 512 * qs + 64 * c + 64)
                res_c[rows] = hs[b, toks, :] + bo_f
        in_maps.append({
            "hsT": hsT,
            "wq": pack_pairs(np.asarray(Wq, np.float32)[:, cols], 128, 128),
            "wk": pack_pairs(np.asarray(Wk, np.float32)[:, cols], 128, 128),
            "wv": pack_pairs(wv_c, 130, 144),
            "wo": Wo_h,
            "bqk": np.ascontiguousarray(bqk_c),
            "bvb": bvb_c,
            "onesb": ones64,
            "res": np.ascontiguousarray(res_c).astype(bf16),
        })
    return in_maps


def _run(inputs, trace=False, trace_kwargs=None):
    if "nc" not in _CACHE:
        _CACHE["nc"] = _build()
    nc = _CACHE["nc"]
    in_maps = _prep_inputs(**inputs)
    r = run_bass_kernel_spmd(nc, in_maps, core_ids=list(range(N_CORES)),
                             trace=trace, **(trace_kwargs or {}))
    full = np.empty((B, S, C), np.float32)
    for c in range(N_CORES):
        for b in range(2):
            o = r.results[c]["out1" if b == 0 else "out2"]
            for qs in range(4):
                full[b, 512 * qs + 64 * c:512 * qs + 64 * c + 64, :] = \
                    o[64 * qs:64 * qs + 64]
    return full, r


def kernel(**inputs):
    full, _ = _run(inputs, trace=False)
    return full


# revision 33
# speedup vs baseline: 1.0558x; 1.0183x over previous
"""Trainium2 Bass kernel for AttnProcessor self-attention (B=2,S=2048,C=1024,H=16).

Sharding: 8 cores, core c owns heads (2c, 2c+1) for both batches (tensor
parallel on the head dim for QKV). Projections run in fp8e4 DoubleRow
(hs and x32-scaled weights; the ones-column carries 32.0 so softmax
normalization absorbs the scale); QK runs bf16; PV runs fp8 DoubleRow
(probs e5m2, v e4m3); output projection bf16.

Token->core output mapping is interleaved so each 512-token q-slice (qs)
contains one 64-token block for every destination core: core c owns tokens
512*qs + 64*c .. +64 of every (b, qs). Attention output is NORMALIZED
SENDER-SIDE (reciprocal of the softmax sums is broadcast across partitions
via a tiny K=1 matmul, then fused into the psum->bf16 drain cast), so each
(b, qs) ships a [8,128,64] bf16 AllToAll right after its PV completes and
the receive side is a single DMA straight into the outproj input tile.
Output projection runs per qs-pair as PE fill work inside later attention
slices / the tail.

b0's hs arrives in 512-token column slices so the projection pipeline
starts after ~1/4 of the load. ScalarE runs ONLY exps; DMAs are issued
from sync/vector/gpsimd rings; DVE does casts/recips/biases/normalization.
"""
import numpy as np

import concourse.bacc as bacc
import concourse.bass as bass
import concourse.tile as tile
import concourse.tile_rust as tile_rust
from concourse import mybir
from concourse.bass_utils import run_bass_kernel_spmd

F32 = mybir.dt.float32
BF16 = mybir.dt.bfloat16
FP8E4 = mybir.dt.float8e4
FP8E5 = mybir.dt.float8e5

B, S, C, H, D = 2, 2048, 1024, 16, 64
N_CORES = 8
BS = B * S  # 4096
SCALE = 1.0 / np.sqrt(D)
WS = 32.0  # fp8 projection weight scale (weights are sigma=1/32)

# a2a chunk geometry (per destination core): 128 rows x 64 tokens bf16
CH_BF = 128 * 64          # bf16 elems per dest chunk (8192)

_CACHE = {}
DR = mybir.MatmulPerfMode.DoubleRow


def _build():
    nc = bacc.Bacc(num_devices=N_CORES)
    hsT = nc.declare_dram_parameter("hsT", [C, BS], FP8E4, isOutput=False)
    wq = nc.declare_dram_parameter("wq", [128, 1024], FP8E4, isOutput=False)
    wk = nc.declare_dram_parameter("wk", [128, 1024], FP8E4, isOutput=False)
    wv = nc.declare_dram_parameter("wv", [128, 1152], FP8E4, isOutput=False)
    wo8 = nc.declare_dram_parameter("wo8", [128, 8192], FP8E4,
                                    isOutput=False)
    bqk = nc.declare_dram_parameter("bqk", [128, 2], F32, isOutput=False)
    bvb = nc.declare_dram_parameter("bvb", [1, 130], F32, isOutput=False)
    onesb = nc.declare_dram_parameter("onesb", [1, 64], BF16, isOutput=False)
    res = nc.declare_dram_parameter("res", [512, C], BF16, isOutput=False)
    out1 = nc.declare_dram_parameter("out1", [256, C], F32, isOutput=True)
    out2 = nc.declare_dram_parameter("out2", [256, C], F32, isOutput=True)

    with tile.TileContext(nc) as tc:
        with (
            tc.tile_pool(name="wpool", bufs=1) as wpool,
            tc.tile_pool(name="hpool", bufs=1) as hpool,
            tc.tile_pool(name="qkpool", bufs=2) as qkpool,
            tc.tile_pool(name="ppool", bufs=4) as ppool,
            tc.tile_pool(name="spool", bufs=4) as spool,
            tc.tile_pool(name="opool", bufs=2) as opool,
            tc.tile_pool(name="psum", bufs=1, space="PSUM") as psum,
            tc.tile_pool(name="dram", bufs=1, space="DRAM") as dram,
        ):
            # ---- weight / constant loads ----
            wq_sb = wpool.tile([128, 1024], FP8E4, tag="wq")
            nc.sync.dma_start(out=wq_sb[:], in_=wq[:])
            wk_sb = wpool.tile([128, 1024], FP8E4, tag="wk")
            nc.scalar.dma_start(out=wk_sb[:], in_=wk[:])
            wv_sb = wpool.tile([128, 1152], FP8E4, tag="wv")
            nc.gpsimd.dma_start(out=wv_sb[:], in_=wv[:])
            bqk_sb = wpool.tile([128, 2], F32, tag="bqk")
            nc.gpsimd.dma_start(out=bqk_sb[:], in_=bqk[:])
            ones_sb = wpool.tile([1, 64], BF16, tag="ones")
            nc.sync.dma_start(out=ones_sb[:], in_=onesb[:])
            bm2_sb = wpool.tile([128, 1], F32, tag="bm2")
            nc.vector.memset(bm2_sb[:], -2.0)
            bvb_sb = wpool.tile([128, 130], F32, tag="bvb")
            bvb_ap = bvb[:]
            nc.gpsimd.dma_start(
                out=bvb_sb[:],
                in_=bass.AP(tensor=bvb_ap.tensor, offset=bvb_ap.offset,
                            ap=[[0, 128], [1, 130]]))

            # b0 hs: chunk-pair tiles [128, 2x512] per (mp, j4), one DMA
            # each, streamed token-slice-major across 3 rings
            rings = [nc.sync, nc.scalar, nc.gpsimd]
            hsT_ap = hsT[:]
            hs0 = [[None] * 4 for _ in range(4)]
            hs0_dmas = []
            n = 0
            for j4 in range(4):
                for mp in range(4):
                    t = hpool.tile([128, 1024], FP8E4, tag=f"hs0_{mp}_{j4}",
                                   name=f"hs0_{mp}_{j4}")
                    for i in range(2):
                        d = rings[n % 3].dma_start(
                            out=t[:, 512 * i:512 * (i + 1)],
                            in_=hsT[128 * (2 * mp + i):
                                    128 * (2 * mp + i + 1),
                                    512 * j4:512 * (j4 + 1)])
                        hs0_dmas.append(d)
                        n += 1
                    hs0[mp][j4] = t
            hs1 = []
            for mp in range(4):
                t = hpool.tile([128, 4096], FP8E4, tag=f"hs1_{mp}",
                               name=f"hs1_{mp}")
                tp = t[:]
                d = [nc.sync, nc.scalar][mp % 2].dma_start(
                    out=bass.AP(tensor=tp.tensor, offset=tp.offset,
                                ap=[list(tp.ap[0]), [2048, 2], [1, 2048]]),
                    in_=bass.AP(tensor=hsT_ap.tensor,
                                offset=hsT_ap.offset
                                + 128 * 2 * mp * BS + 2048,
                                ap=[[BS, 128], [128 * BS, 2], [1, 2048]]))
                for a in hs0_dmas[-3:]:
                    tile_rust.add_dep_helper(
                        d.ins, a.ins, True, "hs1 after hs0 (bandwidth)")
                hs1.append(t)

            # declared bf16 for the collective (CC path dislikes fp8);
            # payload/recv DMAs use fp8 bitcast APs over the same bytes
            a2a_in = [[dram.tile([8, 64, 64], BF16, name=f"a2ain{b}_{qs}")
                       for qs in range(4)] for b in range(2)]
            a2a_out = [[dram.tile([8, 64, 64], BF16,
                                  name=f"a2aout{b}_{qs}")
                        for qs in range(4)] for b in range(2)]

            qT, kT, vS = {}, {}, {}
            _dr = {}

            def hs_rhs(b, mp, col, width):
                """fp8 DR AP for hs chunk-pair mp, token cols [col,col+w)."""
                if b == 0:
                    t = hs0[mp][col // 512][:]
                    off = col % 512
                    return bass.AP(tensor=t.tensor, offset=t.offset + off,
                                   ap=[list(t.ap[0]), [512, 2], [1, width]])
                t = hs1[mp][:]
                return bass.AP(tensor=t.tensor, offset=t.offset + col,
                               ap=[list(t.ap[0]), [2048, 2], [1, width]])

            def emit_proj_qk(b, t_idx, j2):
                """One unit: tensor t_idx (0=q,1=k), one 256-wide s-slice.
                fp8 DoubleRow over chunk pairs."""
                if t_idx == 0:
                    if b not in qT:
                        qT[b] = qkpool.tile([128, 2048], BF16, tag="qT",
                                            name=f"qT{b}")
                    dst, w_sb = qT[b], wq_sb
                else:
                    if b not in kT:
                        kT[b] = qkpool.tile([128, 2048], BF16, tag="kT",
                                            name=f"kT{b}")
                    dst, w_sb = kT[b], wk_sb
                ps = psum.tile([128, 512], F32, tag="aux", bufs=2,
                               name=f"pqk{b}_{t_idx}_{j2}")
                sl = ps[:, 0:256]
                wap = w_sb[:]
                for mp in range(4):
                    nc.tensor.matmul(
                        sl,
                        bass.AP(tensor=wap.tensor,
                                offset=wap.offset + 256 * mp,
                                ap=[list(wap.ap[0]), [128, 2], [1, 128]]),
                        hs_rhs(b, mp, 256 * j2, 256),
                        start=(mp == 0), stop=(mp == 3), perf_mode=DR)
                nc.vector.tensor_scalar_add(
                    out=dst[:, 256 * j2:256 * (j2 + 1)], in0=sl,
                    scalar1=bqk_sb[:, t_idx:t_idx + 1])

            def emit_proj_v(b, i):
                """One unit: one 128-row v' s-tile i -> fp8e4 vS (x32).
                vS layout (PV DoubleRow pairs): pair kc'=i//2 block at
                320*kc', head h at +160*h, parity i%2 at +80."""
                if b not in vS:
                    vS[b] = qkpool.tile([128, 2560], FP8E4, tag="vS",
                                        name=f"vS{b}")
                ps = psum.tile([128, 512], F32, tag="aux", bufs=2,
                               name=f"pv{b}_{i}")
                sl = ps[:, 0:144]
                wap = wv_sb[:]
                for mp in range(4):
                    nc.tensor.matmul(
                        sl,
                        hs_rhs(b, mp, 128 * i, 128),
                        bass.AP(tensor=wap.tensor,
                                offset=wap.offset + 288 * mp,
                                ap=[list(wap.ap[0]), [144, 2], [1, 144]]),
                        start=(mp == 0), stop=(mp == 3), perf_mode=DR)
                vt = vS[b][:]
                bvt = bvb_sb[:]
                slb = ps[:, 0:130]
                nc.vector.tensor_tensor(
                    out=bass.AP(tensor=vt.tensor, offset=vt.offset
                                + 320 * (i // 2) + 80 * (i % 2),
                                ap=[list(vt.ap[0]), [160, 2], [1, 65]]),
                    in0=bass.AP(tensor=slb.tensor, offset=slb.offset,
                                ap=[list(slb.ap[0]), [65, 2], [1, 65]]),
                    in1=bass.AP(tensor=bvt.tensor, offset=bvt.offset,
                                ap=[list(bvt.ap[0]), [65, 2], [1, 65]]),
                    op=mybir.AluOpType.add)

            def emit_attention_qs(b, qs, fill_work):
                """One q-slice (512 q) for both heads; 16 kc steps.
                Per step: 2 fills, exp(kc), QK(kc+1); PV (DoubleRow,
                paired kc blocks) after odd kc."""
                accA = psum.tile([65, 512], F32, tag="accA", bufs=1,
                                 name=f"accA_{b}_{qs}")
                accB = psum.tile([65, 512], F32, tag="accB", bufs=1,
                                 name=f"accB_{b}_{qs}")
                sc_t = {}

                def emit_qk(kc):
                    sc = psum.tile([128, 1024], F32, tag="sc", bufs=2,
                                   name=f"sc_{b}_{qs}_{kc}")
                    sc_t[kc] = sc
                    nc.tensor.matmul(
                        sc[:, 0:512],
                        kT[b][0:64, 128 * kc:128 * (kc + 1)],
                        qT[b][0:64, 512 * qs:512 * (qs + 1)],
                        start=True, stop=True)
                    nc.tensor.matmul(
                        sc[:, 512:1024],
                        kT[b][64:128, 128 * kc:128 * (kc + 1)],
                        qT[b][64:128, 512 * qs:512 * (qs + 1)],
                        start=True, stop=True)

                emit_qk(0)
                pr2 = None
                for kc in range(16):
                    for _ in range(2):
                        if fill_work:
                            fill_work.pop(0)()
                    if kc % 2 == 0:
                        pr2 = ppool.tile([128, 2048], FP8E5, tag="pr",
                                         bufs=2, name=f"pr_{b}_{qs}_{kc}")
                    nc.scalar.activation(
                        pr2[:, 1024 * (kc % 2):1024 * (kc % 2 + 1)],
                        sc_t.pop(kc)[:],
                        mybir.ActivationFunctionType.Exp,
                        scale=float(SCALE / (WS * WS)))
                    if kc < 15:
                        emit_qk(kc + 1)
                    if kc % 2 == 1:
                        kp = kc // 2
                        vt = vS[b][:]
                        prt = pr2[:]
                        for h, acc in ((0, accA), (1, accB)):
                            nc.tensor.matmul(
                                acc[:],
                                bass.AP(tensor=vt.tensor, offset=vt.offset
                                        + 320 * kp + 160 * h,
                                        ap=[list(vt.ap[0]), [80, 2],
                                            [1, 65]]),
                                bass.AP(tensor=prt.tensor,
                                        offset=prt.offset + 512 * h,
                                        ap=[list(prt.ap[0]), [1024, 2],
                                            [1, 512]]),
                                start=(kp == 0), stop=(kp == 7),
                                perf_mode=DR)

                # drain (returned as deferred closures so the PE-queue
                # bcast matmul never stalls the next slice's QK stream):
                # reciprocal of sums (bf16), broadcast across 64 partitions
                # via K=1 matmul, normalize+cast in one DVE op per head,
                # ship per-dest chunks.
                def drain0(accA=accA, accB=accB, b=b, qs=qs):
                    sm2 = spool.tile([1, 1024], F32, tag="sm2",
                                     name=f"sm2_{b}_{qs}")
                    nc.vector.tensor_copy(sm2[:, 0:512], accA[64:65, :])
                    nc.vector.tensor_copy(sm2[:, 512:1024], accB[64:65, :])
                    rb2f = spool.tile([1, 1024], F32, tag="rbf",
                                      name=f"rbf_{b}_{qs}")
                    nc.vector.reciprocal_approx_fast(rb2f[:], sm2[:])
                    rb2 = spool.tile([1, 1024], BF16, tag="rb",
                                     name=f"rb_{b}_{qs}")
                    nc.vector.tensor_copy(rb2[:], rb2f[:])
                    bc = psum.tile([128, 1024], F32, tag="sc", bufs=2,
                                   name=f"bc_{b}_{qs}")
                    nc.tensor.matmul(bc[0:64, 0:512], ones_sb[:],
                                     rb2[:, 0:512], start=True, stop=True)
                    nc.tensor.matmul(bc[0:64, 512:1024], ones_sb[:],
                                     rb2[:, 512:1024], start=True, stop=True)
                    bcb = spool.tile([64, 1024], BF16, tag="bcb",
                                     name=f"bcb_{b}_{qs}")
                    nc.vector.tensor_copy(bcb[:], bc[0:64, :])
                    st = spool.tile([64, 1024], FP8E4, tag="st",
                                    name=f"st_{b}_{qs}")
                    _dr[(b, qs)] = (bcb, st)

                def drain_h(h, acc, b=b, qs=qs):
                    bcb, st = _dr[(b, qs)]
                    a2a_t = a2a_in[b][qs][:].bitcast(FP8E4)
                    stp = st[:]
                    peng = nc.gpsimd if b == 0 else nc.sync
                    nc.vector.tensor_tensor(
                        out=st[:, 512 * h:512 * (h + 1)],
                        in0=acc[0:64, :],
                        in1=bcb[:, 512 * h:512 * (h + 1)],
                        op=mybir.AluOpType.mult)
                    # payload: st[0:64, 512h + 64*d+t] -> a2a[d, 64h+r, t]
                    peng.dma_start(
                        out=bass.AP(tensor=a2a_t.tensor, offset=a2a_t.offset
                                    + 64 * h * 64,
                                    ap=[[64, 64], [CH_BF, 8], [1, 64]]),
                        in_=bass.AP(tensor=stp.tensor,
                                    offset=stp.offset + 512 * h,
                                    ap=[list(stp.ap[0]), [64, 8], [1, 64]]))
                    if h == 1:
                        nc.gpsimd.collective_compute(
                            "AllToAll", mybir.AluOpType.bypass,
                            replica_groups=[list(range(8))],
                            ins=[a2a_in[b][qs][:]],
                            outs=[a2a_out[b][qs][:]])

                return [drain0,
                        lambda: drain_h(0, accA),
                        lambda: drain_h(1, accB)]

            # ---- output side ----
            an_all = {}

            def emit_recv(b, p, half):
                """After A2A (b, qs=2p+half): one DMA into the an tile."""
                qs = 2 * p + half
                a2a_t = a2a_out[b][qs][:].bitcast(FP8E4)
                if (b, p) not in an_all:
                    an_all[(b, p)] = opool.tile([128, 1024], FP8E4,
                                                tag="an", name=f"an{b}_{p}")
                anap = an_all[(b, p)][:]
                raw_d = nc.sync.dma_start(
                    out=bass.AP(tensor=anap.tensor,
                                offset=anap.offset + 64 * half,
                                ap=[list(anap.ap[0]), [128, 8], [1, 64]]),
                    in_=bass.AP(tensor=a2a_t.tensor, offset=a2a_t.offset,
                                ap=[[64, 128], [CH_BF, 8], [1, 64]]))
                return raw_d

            res_sb = []

            out_ps = {}

            def emit_out_mm(b, p, co):
                """Outproj half matmuls (fp8 DoubleRow, x32*x32 scaled):
                512 cols for 128 tokens of pair."""
                anap = an_all[(b, p)][:]
                w8 = wo8_sb[:]
                ps = psum.tile([128, 512], F32, tag="aux", bufs=2,
                               name=f"op{b}_{p}_{co}")
                for jp in range(4):
                    nc.tensor.matmul(
                        ps[:],
                        bass.AP(tensor=anap.tensor,
                                offset=anap.offset + 256 * jp,
                                ap=[list(anap.ap[0]), [128, 2], [1, 128]]),
                        bass.AP(tensor=w8.tensor,
                                offset=w8.offset + 2048 * jp + 512 * co,
                                ap=[list(w8.ap[0]), [1024, 2], [1, 512]]),
                        start=(jp == 0), stop=(jp == 3), perf_mode=DR)
                out_ps[(b, p, co)] = ps

            def emit_out_fin(b, p, co):
                """Residual add + store (DVE+sync). Emitted late so the
                psum read never sits ahead of drain casts in the DVE FIFO."""
                ps = out_ps.pop((b, p, co))
                ob = opool.tile([128, 512], F32, tag="ob",
                                name=f"ob{b}_{p}_{co}")
                nc.vector.scalar_tensor_tensor(
                    out=ob[:], in0=ps[:], scalar=float(1.0 / (WS * WS)),
                    in1=res_sb[2 * b + p][:, 512 * co:512 * (co + 1)],
                    op0=mybir.AluOpType.mult, op1=mybir.AluOpType.add)
                out_t = out1 if b == 0 else out2
                nc.sync.dma_start(
                    out=out_t[128 * p:128 * (p + 1),
                              512 * co:512 * (co + 1)],
                    in_=ob[:])

            # ---------------- emission ----------------
            emit_proj_qk(0, 0, 0)
            emit_proj_qk(0, 0, 1)
            emit_proj_qk(0, 1, 0)
            emit_proj_v(0, 0)
            emit_proj_v(0, 1)

            def qk_u(b, t, j2):
                return lambda: emit_proj_qk(b, t, j2)

            def v_u(b, i):
                return lambda: emit_proj_v(b, i)

            def nop():
                pass

            fill = [qk_u(0, 1, 1), v_u(0, 2),
                    qk_u(0, 1, 2), v_u(0, 3),
                    qk_u(0, 1, 3), v_u(0, 4),
                    qk_u(0, 1, 4), v_u(0, 5),
                    qk_u(0, 1, 5), v_u(0, 6),
                    qk_u(0, 1, 6), v_u(0, 7),
                    qk_u(0, 1, 7), v_u(0, 8),
                    v_u(0, 9), v_u(0, 10),
                    v_u(0, 11), v_u(0, 12),
                    v_u(0, 13), v_u(0, 14),
                    v_u(0, 15), qk_u(0, 0, 2),
                    qk_u(0, 0, 3)]
            dr = emit_attention_qs(0, 0, fill)
            assert not fill

            # wo / res load (sync queue)
            wo8_sb = hpool.tile([128, 8192], FP8E4, tag="wo8", name="wo8")
            nc.sync.dma_start(out=wo8_sb[:, 0:4096], in_=wo8[:, 0:4096])
            nc.sync.dma_start(out=wo8_sb[:, 4096:8192],
                              in_=wo8[:, 4096:8192])
            for st_i in range(4):
                t = wpool.tile([128, 1024], BF16, tag=f"res{st_i}",
                               name=f"res{st_i}")
                nc.sync.dma_start(out=t[:],
                                  in_=res[128 * st_i:128 * (st_i + 1), :])
                res_sb.append(t)

            # b1 projections fill b0 qs1-qs3 (hs1 lands ~35us in)
            fill = dr + [qk_u(0, 0, 4), qk_u(0, 0, 5)]
            for j2 in range(8):
                fill.append(qk_u(1, 1, j2))
            fill += [v_u(1, 0), v_u(1, 1)]
            dr = emit_attention_qs(0, 1, fill)
            fill = dr + [qk_u(0, 0, 6), qk_u(0, 0, 7)]
            for i in range(2, 8):
                fill.append(v_u(1, i))
            fill += [qk_u(1, 0, 0), qk_u(1, 0, 1), qk_u(1, 0, 2),
                     qk_u(1, 0, 3)]
            dr = emit_attention_qs(0, 2, fill)
            fill = dr + [qk_u(1, 0, 4), qk_u(1, 0, 5), qk_u(1, 0, 6),
                         qk_u(1, 0, 7)]
            for i in range(8, 16):
                fill.append(v_u(1, i))
            dr = emit_attention_qs(0, 3, fill)

            dr = emit_attention_qs(1, 0, list(dr))
            dr = emit_attention_qs(1, 1, list(dr))
            emit_recv(0, 0, 0)
            emit_recv(0, 0, 1)
            fill = dr + [nop] * 8 + [lambda: emit_out_mm(0, 0, 0),
                                     lambda: emit_out_mm(0, 0, 1)]
            dr = emit_attention_qs(1, 2, fill)
            emit_out_fin(0, 0, 0)
            emit_out_fin(0, 0, 1)
            emit_recv(0, 1, 0)
            emit_recv(0, 1, 1)
            fill = dr + [nop] * 8 + [lambda: emit_out_mm(0, 1, 0),
                                     lambda: emit_out_mm(0, 1, 1)]
            dr = emit_attention_qs(1, 3, fill)
            # last drain runs immediately: its A2A is the critical path
            for f in dr:
                f()
            emit_out_fin(0, 1, 0)
            emit_out_fin(0, 1, 1)
            # tail: out(1,0) doubles as PE warm-keeper during last A2A
            emit_recv(1, 0, 0)
            emit_recv(1, 0, 1)
            emit_out_mm(1, 0, 0)
            emit_out_fin(1, 0, 0)
            emit_out_mm(1, 0, 1)
            emit_out_fin(1, 0, 1)
            raw_d = emit_recv(1, 1, 0)
            warm = psum.tile([128, 512], F32, tag="aux", bufs=2,
                             name="warm")
            for wi in range(6):
                w = nc.tensor.matmul(warm[:], wo8_sb[:, 0:128],
                                     wo8_sb[:, 1024:1536],
                                     start=True, stop=True,
                                     skip_group_check=True)
                if wi == 0:
                    tile_rust.add_dep_helper(
                        w.ins, raw_d.ins, True, "warm PE near last recv")
            emit_recv(1, 1, 1)
            emit_out_mm(1, 1, 0)
            emit_out_fin(1, 1, 0)
            emit_out_mm(1, 1, 1)
            emit_out_fin(1, 1, 1)
    nc.finalize()
    return nc


def _prep_inputs(hidden_states, Wq, bq, Wk, bk, Wv, bv, Wo, bo):
    import ml_dtypes
    bf16 = ml_dtypes.bfloat16
    fp8 = ml_dtypes.float8_e4m3fn
    hs = np.asarray(hidden_states, np.float32)
    hsT = np.clip(np.ascontiguousarray(
        hs.transpose(2, 0, 1).reshape(C, BS)), -240, 240).astype(fp8)
    Wof = np.asarray(Wo, np.float32)
    wo8_h = np.zeros((128, 8192), np.float32)
    for j in range(8):
        jp, e = j // 2, j % 2
        wo8_h[:, 2048 * jp + 1024 * e:2048 * jp + 1024 * e + 1024] = \
            WS * Wof[128 * j:128 * (j + 1), :]
    wo8_h = np.clip(wo8_h, -240, 240).astype(fp8)
    bo_f = np.asarray(bo, np.float32)
    ones64 = np.ones((1, 64), np.float32).astype(bf16)

    def pack_pairs(w, ncols, stride):
        """[C, ncols] -> [128, 8*stride]: col 2*stride*mp + stride*i + m
        = WS * w[128*(2*mp+i) + p, m], fp8."""
        out = np.zeros((128, 8 * stride), np.float32)
        for mp in range(4):
            for i in range(2):
                blk = w[128 * (2 * mp + i):128 * (2 * mp + i + 1), :]
                out[:, 2 * stride * mp + stride * i:
                    2 * stride * mp + stride * i + ncols] = WS * blk
        return np.clip(out, -240, 240).astype(fp8)

    in_maps = []
    for c in range(N_CORES):
        h0 = 2 * c
        cols = slice(64 * h0, 64 * h0 + 128)
        wv_c = np.zeros((C, 130), np.float32)
        bvb_c = np.zeros((1, 130), np.float32)
        for a in range(2):
            hd = slice(64 * (h0 + a), 64 * (h0 + a + 1))
            wv_c[:, 65 * a:65 * a + 64] = np.asarray(Wv, np.float32)[:, hd]
            bvb_c[0, 65 * a:65 * a + 64] = WS * np.asarray(
                bv, np.float32)[hd]
            # ones column 1.0 (not WS): acc[0:64]/acc[64] then lands at
            # 32x the attention output, exactly the fp8 payload scale.
            bvb_c[0, 65 * a + 64] = 1.0
        bqk_c = WS * np.stack([np.asarray(bq, np.float32)[cols],
                               np.asarray(bk, np.float32)[cols]], axis=1)
        res_c = np.empty((512, C), np.float32)
        for b in range(2):
            for qs in range(4):
                rows = slice(64 * (4 * b + qs), 64 * (4 * b + qs) + 64)
                toks = slice(512 * qs + 64 * c, 512 * qs + 64 * c + 64)
                res_c[rows] = hs[b, toks, :] + bo_f
        in_maps.append({
            "hsT": hsT,
            "wq": pack_pairs(np.asarray(Wq, np.float32)[:, cols], 128, 128),
            "wk": pack_pairs(np.asarray(Wk, np.float32)[:, cols], 128, 128),
            "wv": pack_pairs(wv_c, 130, 144),
            "wo8": wo8_h,
            "bqk": np.ascontiguousarray(bqk_c),
            "bvb": bvb_c,
            "onesb": ones64,
            "res": np.ascontiguousarray(res_c).astype(bf16),
        })
    return in_maps


def _run(inputs, trace=False, trace_kwargs=None):
    if "nc" not in _CACHE:
        _CACHE["nc"] = _build()
    nc = _CACHE["nc"]
    in_maps = _prep_inputs(**inputs)
    r = run_bass_kernel_spmd(nc, in_maps, core_ids=list(range(N_CORES)),
                             trace=trace, **(trace_kwargs or {}))
    full = np.empty((B, S, C), np.float32)
    for c in range(N_CORES):
        for b in range(2):
            o = r.results[c]["out1" if b == 0 else "out2"]
            for qs in range(4):
                full[b, 512 * qs + 64 * c:512 * qs + 64 * c + 64, :] = \
                    o[64 * qs:64 * qs + 64]
    return full, r


def kernel(**inputs):
    full, _ = _run(inputs, trace=False)
    return full
